# revision 2
# baseline (speedup 1.0000x reference)
"""Contrastive loss (GRACE-style semi_loss pair) on 8 trn2 NeuronCores.

Math (reference):
    a = z1 / ||z1||_row ; b = z2 / ||z2||_row         (N=8192, D=512)
    refl    = exp(a @ a.T / tau) ; between = exp(a @ b.T / tau)
    l1_i = -log(between_ii / (refl.sum(1) + between.sum(1) - refl_ii))
    l2   = same with (z2, z1) swapped
    loss = mean(0.5 * (l1 + l2))

Key identities used:
  - between2 (for l2) = between.T, so its row sums are COLUMN sums of
    exp(a@b.T/tau) -> one cross-core ReduceScatter of [8192] floats,
    no 4th matmul.
  - refl_ii = exp(1/tau) exactly (rows are unit-norm).
  - between_ii needs only dab_i = a_i . b_i (computed row-wise in fp32).
  - l1_i = log(denom1_i) - dab_i/tau ; l2_i = log(denom2_i) - dab_i/tau.

Sharding: data-parallel rows. Every core receives the full z (row-major,
for norms) and full zT (d-major, the matmul moving operand) plus its own
1024-row slice (stationary side). Per core, pipelined per 512-column
chunk:
  prep:  stream z row-major, fused square+row-sum on DVE; 1/sqrt via
         Newton iteration on DVE (rows of randn(512) have norm 22.6+-3%,
         so a constant seed converges in 3 steps -- no ACT table sets,
         no serial norm phase); bf16 1/norm -> DRAM -> stride-0
         broadcast DMA -> [128,512]; scale streamed zT tiles on DVE into
         persistent bf16 operands.
  main:  per (chunk n, local 128-row block m): 3 accumulation groups of
         4 bf16 matmuls (S_aa, S_ab, S_bb), fused exp+row-sum on ACT
         (aa/bb exp'd in place in PSUM), exp(S_ab) kept bf16; column
         sums accumulated bf16 on DVE, folded once per chunk by a
         ones-matmul.
  tail:  ReduceScatter(column sums), denominators, Ln, per-core partial
         -> AllReduce scalar -> loss.
"""

import numpy as np
from contextlib import ExitStack

import concourse.bass as bass
import concourse.tile as tile
from concourse import bacc, mybir
from concourse.bass_utils import run_bass_kernel_spmd

N = 8192
D = 512
P = 128
NCORES = 8
LOCAL = N // NCORES            # 1024 rows per core
M_CH = LOCAL // P              # 8 local row chunks of 128
N_CH = N // 512                # 16 column chunks of 512
KC = D // P                    # 4 contraction chunks of 128
TAU = 0.4
EXPD = float(np.exp(1.0 / TAU))   # diagonal of exp(S_aa/tau): rows unit-norm
Y0 = float(D) ** -0.5             # Newton rsqrt seed: sumsq ~ 512 +- 6%

FP32 = mybir.dt.float32
BF16 = mybir.dt.bfloat16
ALU = mybir.AluOpType
ACTF = mybir.ActivationFunctionType


def _build():
    nc = bacc.Bacc("TRN2", debug=False, num_devices=NCORES)
    z1 = nc.dram_tensor("z1", [N, D], FP32, kind="ExternalInput").ap()
    z2 = nc.dram_tensor("z2", [N, D], FP32, kind="ExternalInput").ap()
    z1T = nc.dram_tensor("z1T", [D, N], FP32, kind="ExternalInput").ap()
    z2T = nc.dram_tensor("z2T", [D, N], FP32, kind="ExternalInput").ap()
    z1l = nc.dram_tensor("z1l", [LOCAL, D], FP32, kind="ExternalInput").ap()
    z2l = nc.dram_tensor("z2l", [LOCAL, D], FP32, kind="ExternalInput").ap()
    z1lT = nc.dram_tensor("z1lT", [D, LOCAL], FP32, kind="ExternalInput").ap()
    z2lT = nc.dram_tensor("z2lT", [D, LOCAL], FP32, kind="ExternalInput").ap()
    loss = nc.dram_tensor("loss", [1, 1], FP32, kind="ExternalOutput").ap()

    with tile.TileContext(nc) as tc, ExitStack() as ctx:
        big = ctx.enter_context(tc.tile_pool(name="big", bufs=1))
        stage = ctx.enter_context(tc.tile_pool(name="stage", bufs=16))
        small = ctx.enter_context(tc.tile_pool(name="small", bufs=1))
        scratch = ctx.enter_context(tc.tile_pool(name="scratch", bufs=2))
        pmm = ctx.enter_context(tc.tile_pool(name="pmm", bufs=4, space="PSUM"))
        pbc = ctx.enter_context(tc.tile_pool(name="pbc", bufs=2, space="PSUM"))
        pcol = ctx.enter_context(tc.tile_pool(name="pcol", bufs=2, space="PSUM"))
        dram = ctx.enter_context(tc.tile_pool(name="dram", bufs=1, space="DRAM"))

        # ---- constants --------------------------------------------------
        ones_bf = small.tile([P, 1], BF16, tag="ones_bf", name="ones_bf")
        nc.vector.memset(ones_bf, 1.0)
        ones_f32 = small.tile([P, 1], FP32, tag="ones_f32", name="ones_f32")
        nc.vector.memset(ones_f32, 1.0)
        ones_row = small.tile([1, P], BF16, tag="ones_row", name="ones_row")
        nc.vector.memset(ones_row, 1.0)

        # ---- persistent operands ---------------------------------------
        ATL1 = big.tile([P, KC, LOCAL], BF16, tag="ATL1", name="ATL1")
        ATL2 = big.tile([P, KC, LOCAL], BF16, tag="ATL2", name="ATL2")
        # AT chunk operands live only from scale(n) to main(n): rotate 4-deep
        AT1 = {}
        AT2 = {}
        invnb_l1 = big.tile([P, LOCAL], BF16, tag="invnb_l1", name="invnb_l1")
        invnb_l2 = big.tile([P, LOCAL], BF16, tag="invnb_l2", name="invnb_l2")

        rsp_aa = [
            small.tile([P, N_CH], FP32, tag=f"rsp_aa{m}", name=f"rsp_aa{m}")
            for m in range(M_CH)
        ]
        rsp_ab = [
            small.tile([P, N_CH], FP32, tag=f"rsp_ab{m}", name=f"rsp_ab{m}")
            for m in range(M_CH)
        ]
        rsp_bb = [
            small.tile([P, N_CH], FP32, tag=f"rsp_bb{m}", name=f"rsp_bb{m}")
            for m in range(M_CH)
        ]

        ss_l1 = small.tile([P, M_CH], FP32, tag="ss_l1", name="ss_l1")
        ss_l2 = small.tile([P, M_CH], FP32, tag="ss_l2", name="ss_l2")
        u_ab = small.tile([P, M_CH], FP32, tag="u_ab", name="u_ab")

        # collective buffers
        cc1_in = dram.tile([1, N], FP32, tag="cc1_in", name="cc1_in")
        cc1_out = dram.tile([M_CH, P], FP32, tag="cc1_out", name="cc1_out")
        cc2_in = dram.tile([1, 1], FP32, tag="cc2_in", name="cc2_in")
        cc2_out = dram.tile(
            [1, 1], FP32, tag="cc2_out", name="cc2_out", addr_space="Shared"
        )

        def sumsq(zt, acc_slice, nm, other=None):
            # fused (zt * 1.0) * other with row-sum on DVE
            sq = scratch.tile([P, D], BF16, tag="sq", name=f"sq_{nm}")
            nc.vector.scalar_tensor_tensor(
                out=sq, in0=zt, scalar=1.0, in1=other if other is not None else zt,
                op0=ALU.mult, op1=ALU.mult, accum_out=acc_slice,
            )

        def rsqrt_newton(ss, w, nm, iters=3):
            """y ~= 1/sqrt(ss) on DVE only. ss ~ D +- ~6% so constant seed
            Y0=D^-0.5 converges: rel err ~2e-2 -> 6e-4 -> 5e-7."""
            ssh = scratch.tile([P, w], FP32, tag="rq_ssh", name=f"ssh_{nm}")
            nc.vector.tensor_scalar_mul(ssh, ss, 0.5)
            y = scratch.tile([P, w], FP32, tag="rq_y", name=f"y_{nm}")
            # y1 = Y0*(1.5 - ss*0.5*Y0^2) = (-Y0^3)*ssh + 1.5*Y0
            nc.vector.tensor_scalar(
                out=y, in0=ssh, scalar1=-(Y0**3), scalar2=1.5 * Y0,
                op0=ALU.mult, op1=ALU.add,
            )
            t = scratch.tile([P, w], FP32, tag="rq_t", name=f"t_{nm}")
            u = scratch.tile([P, w], FP32, tag="rq_u", name=f"u_{nm}")
            for i in range(iters - 1):
                nc.vector.tensor_mul(t, y, y)          # y^2
                nc.vector.tensor_mul(t, t, ssh)        # 0.5*ss*y^2
                nc.vector.tensor_mul(u, y, t)          # 0.5*ss*y^3
                # y = 1.5*y - u
                nc.vector.scalar_tensor_tensor(
                    out=y, in0=y, scalar=1.5, in1=u,
                    op0=ALU.mult, op1=ALU.subtract,
                )
            return y

        # ---- local rows: norms, dab, stationary operands ----------------
        # z1l+u on DVE, z2l sumsq on ACT (Square is in every table set) so
        # the two engines chew the head-of-kernel chain in parallel.
        lz = []
        for t in range(M_CH):
            zt1 = stage.tile([P, D], FP32, tag="st_z1", name=f"zl1_{t}", bufs=16)
            nc.sync.dma_start(out=zt1, in_=z1l[P * t : P * (t + 1), :])
            zt2 = stage.tile([P, D], FP32, tag="st_z2", name=f"zl2_{t}", bufs=16)
            nc.sync.dma_start(out=zt2, in_=z2l[P * t : P * (t + 1), :])
            lz.append((zt1, zt2))
        for t, (zt1, zt2) in enumerate(lz):
            sumsq(zt1, ss_l1[:, t : t + 1], f"l1_{t}")
            sq2 = scratch.tile([P, D], BF16, tag="sq2", name=f"sq2_{t}")
            nc.scalar.activation(
                out=sq2, in_=zt2, func=ACTF.Square,
                accum_out=ss_l2[:, t : t + 1],
            )
            sumsq(zt1, u_ab[:, t : t + 1], f"u_{t}", other=zt2)

        invn_l1 = rsqrt_newton(ss_l1, M_CH, "l1")
        invn_l2 = rsqrt_newton(ss_l2, M_CH, "l2")

        # dab_i = u_i / (||z1_i|| * ||z2_i||)
        dab = small.tile([P, M_CH], FP32, tag="dab", name="dab")
        nc.vector.tensor_mul(dab, u_ab, invn_l1)
        nc.vector.tensor_mul(dab, dab, invn_l2)

        # 1/norm -> DRAM flat (row order) -> stride-0 broadcast back
        ivcl = scratch.tile([P, 2 * M_CH], BF16, tag="ivcl", name="ivcl")
        nc.vector.tensor_copy(ivcl[:, 0:M_CH], invn_l1)
        nc.vector.tensor_copy(ivcl[:, M_CH : 2 * M_CH], invn_l2)
        ivdl1 = dram.tile([1, LOCAL], BF16, tag="ivdl1", name="ivdl1")
        ivdl2 = dram.tile([1, LOCAL], BF16, tag="ivdl2", name="ivdl2")
        nc.scalar.dma_start(
            out=ivdl1.rearrange("o (t p) -> p (o t)", p=P), in_=ivcl[:, 0:M_CH]
        )
        nc.scalar.dma_start(
            out=ivdl2.rearrange("o (t p) -> p (o t)", p=P),
            in_=ivcl[:, M_CH : 2 * M_CH],
        )
        nc.scalar.dma_start(out=invnb_l1, in_=ivdl1.to_broadcast([P, LOCAL]))
        nc.scalar.dma_start(out=invnb_l2, in_=ivdl2.to_broadcast([P, LOCAL]))
        for k in range(KC):
            zlt1 = stage.tile([P, LOCAL], FP32, tag="zlT", name=f"zlT1_{k}")
            nc.sync.dma_start(out=zlt1, in_=z1lT[P * k : P * (k + 1), :])
            nc.vector.tensor_mul(ATL1[:, k, :], zlt1, invnb_l1)
            zlt2 = stage.tile([P, LOCAL], FP32, tag="zlT", name=f"zlT2_{k}")
            nc.sync.dma_start(out=zlt2, in_=z2lT[P * k : P * (k + 1), :])
            nc.vector.tensor_mul(ATL2[:, k, :], zlt2, invnb_l2)

        # ---- per column chunk: norms then operand scaling ---------------
        ivd1 = [
            dram.tile([1, 512], BF16, tag=f"iv1_{n}", name=f"ivd1_{n}")
            for n in range(N_CH)
        ]
        ivd2 = [
            dram.tile([1, 512], BF16, tag=f"iv2_{n}", name=f"ivd2_{n}")
            for n in range(N_CH)
        ]

        def norm_chunk(n):
            # row norms for rows 512n..512(n+1): z1 sumsq on DVE, z2 on ACT
            ssc = scratch.tile([P, 8], FP32, tag="ssc", name=f"ssc_{n}", bufs=4)
            for j in range(4):
                t = 4 * n + j
                zt1 = stage.tile([P, D], FP32, tag="st_z1", name=f"zf1_{t}", bufs=16)
                nc.sync.dma_start(out=zt1, in_=z1[P * t : P * (t + 1), :])
                sumsq(zt1, ssc[:, j : j + 1], f"f1_{t}")
                zt2 = stage.tile([P, D], FP32, tag="st_z2", name=f"zf2_{t}", bufs=16)
                nc.sync.dma_start(out=zt2, in_=z2[P * t : P * (t + 1), :])
                sq2 = scratch.tile([P, D], BF16, tag="sq2", name=f"sqf2_{t}")
                nc.scalar.activation(
                    out=sq2, in_=zt2, func=ACTF.Square,
                    accum_out=ssc[:, 4 + j : 5 + j],
                )
            ivn = rsqrt_newton(ssc, 8, f"f{n}")
            ivc = scratch.tile([P, 8], BF16, tag="ivc", name=f"ivc_{n}", bufs=4)
            nc.vector.tensor_copy(ivc, ivn)
            nc.scalar.dma_start(
                out=ivd1[n].rearrange("o (t p) -> p (o t)", p=P), in_=ivc[:, 0:4]
            )
            nc.scalar.dma_start(
                out=ivd2[n].rearrange("o (t p) -> p (o t)", p=P), in_=ivc[:, 4:8]
            )

        def scale_chunk(n):
            # broadcast 1/norm across partitions with a K=1 bf16 matmul,
            # scale streamed zT tiles on DVE (reading the PSUM broadcast)
            AT1[n] = big.tile(
                [P, KC, 512], BF16, tag="AT1", name=f"AT1_{n}", bufs=4
            )
            AT2[n] = big.tile(
                [P, KC, 512], BF16, tag="AT2", name=f"AT2_{n}", bufs=4
            )
            ivf1 = stage.tile([1, 512], BF16, tag="ivf1", name=f"ivf1_{n}", bufs=4)
            nc.scalar.dma_start(out=ivf1, in_=ivd1[n])
            pb1 = pbc.tile([P, 512], FP32, tag="bc", name=f"pb1_{n}")
            nc.tensor.matmul(pb1, ones_row, ivf1, start=True, stop=True)
            for k in range(KC):
                zt = stage.tile([P, 512], FP32, tag="st_z1", name=f"zT1_{n}_{k}", bufs=16)
                nc.sync.dma_start(
                    out=zt, in_=z1T[P * k : P * (k + 1), 512 * n : 512 * (n + 1)]
                )
                nc.vector.tensor_mul(AT1[n][:, k, :], zt, pb1)
            ivf2 = stage.tile([1, 512], BF16, tag="ivf2", name=f"ivf2_{n}", bufs=4)
            nc.scalar.dma_start(out=ivf2, in_=ivd2[n])
            pb2 = pbc.tile([P, 512], FP32, tag="bc", name=f"pb2_{n}")
            nc.tensor.matmul(pb2, ones_row, ivf2, start=True, stop=True)
            for k in range(KC):
                zt2 = stage.tile([P, 512], FP32, tag="st_z2", name=f"zT2_{n}_{k}", bufs=16)
                nc.sync.dma_start(
                    out=zt2, in_=z2T[P * k : P * (k + 1), 512 * n : 512 * (n + 1)]
                )
                nc.vector.tensor_mul(AT2[n][:, k, :], zt2, pb2)

        def main_chunk(n):
            colacc = scratch.tile(
                [P, 512], BF16, tag="colacc", name=f"colacc_{n}", bufs=2
            )
            for m in range(M_CH):
                aa = pmm.tile([P, 512], FP32, tag="mm", name=f"aa_{n}_{m}")
                ab = pmm.tile([P, 512], FP32, tag="mm", name=f"ab_{n}_{m}")
                bb = pmm.tile([P, 512], FP32, tag="mm", name=f"bb_{n}_{m}")
                for k in range(KC):
                    nc.tensor.matmul(
                        aa, ATL1[:, k, P * m : P * (m + 1)], AT1[n][:, k, :],
                        start=(k == 0), stop=(k == KC - 1),
                    )
                for k in range(KC):
                    nc.tensor.matmul(
                        ab, ATL1[:, k, P * m : P * (m + 1)], AT2[n][:, k, :],
                        start=(k == 0), stop=(k == KC - 1),
                    )
                for k in range(KC):
                    nc.tensor.matmul(
                        bb, ATL2[:, k, P * m : P * (m + 1)], AT2[n][:, k, :],
                        start=(k == 0), stop=(k == KC - 1),
                    )
                nc.scalar.activation(
                    out=aa, in_=aa, func=ACTF.Exp, scale=1.0 / TAU,
                    accum_out=rsp_aa[m][:, n : n + 1],
                )
                exab = scratch.tile(
                    [P, 512], BF16, tag="exab", name=f"exab_{n}_{m}", bufs=3
                )
                nc.scalar.activation(
                    out=exab, in_=ab, func=ACTF.Exp, scale=1.0 / TAU,
                    accum_out=rsp_ab[m][:, n : n + 1],
                )
                nc.scalar.activation(
                    out=bb, in_=bb, func=ACTF.Exp, scale=1.0 / TAU,
                    accum_out=rsp_bb[m][:, n : n + 1],
                )
                # column-sum accumulation on DVE (frees PE, breaks ACT->PE dep)
                if m == 0:
                    nc.vector.tensor_copy(colacc, exab)
                else:
                    nc.vector.tensor_add(colacc, colacc, exab)
            colp = pcol.tile([1, 512], FP32, tag="col", name=f"colp_{n}")
            nc.tensor.matmul(colp, ones_bf, colacc, start=True, stop=True)
            csb = scratch.tile([1, 512], FP32, tag="csb", name=f"csb_{n}", bufs=1)
            nc.vector.tensor_copy(csb, colp)
            nc.scalar.dma_start(out=cc1_in[:, 512 * n : 512 * (n + 1)], in_=csb)

        # software pipeline: norms 4 chunks ahead, operand scaling 2 ahead,
        # so the prep chains sit ahead of main-chunk work in the FIFO queues
        norm_chunk(0)
        scale_chunk(0)
        norm_chunk(1)
        scale_chunk(1)
        norm_chunk(2)
        norm_chunk(3)
        for n in range(N_CH):
            if n + 4 < N_CH:
                norm_chunk(n + 4)
            if n + 2 < N_CH:
                scale_chunk(n + 2)
            main_chunk(n)

        # ---- tail -------------------------------------------------------
        rs_aa = small.tile([P, M_CH], FP32, tag="rs_aa", name="rs_aa")
        rs_ab = small.tile([P, M_CH], FP32, tag="rs_ab", name="rs_ab")
        rs_bb = small.tile([P, M_CH], FP32, tag="rs_bb", name="rs_bb")
        for m in range(M_CH):
            nc.vector.reduce_sum(
                out=rs_aa[:, m : m + 1], in_=rsp_aa[m], axis=mybir.AxisListType.X
            )
            nc.vector.reduce_sum(
                out=rs_ab[:, m : m + 1], in_=rsp_ab[m], axis=mybir.AxisListType.X
            )
            nc.vector.reduce_sum(
                out=rs_bb[:, m : m + 1], in_=rsp_bb[m], axis=mybir.AxisListType.X
            )

        denom1 = small.tile([P, M_CH], FP32, tag="denom1", name="denom1")
        nc.vector.scalar_tensor_tensor(
            out=denom1, in0=rs_aa, scalar=-EXPD, in1=rs_ab,
            op0=ALU.add, op1=ALU.add,
        )

        nc.gpsimd.collective_compute(
            "ReduceScatter",
            ALU.add,
            replica_groups=[list(range(NCORES))],
            ins=[cc1_in.opt()],
            outs=[cc1_out.opt()],
        )
        colsum_l = small.tile([P, M_CH], FP32, tag="colsum_l", name="colsum_l")
        nc.scalar.dma_start(out=colsum_l, in_=cc1_out.rearrange("m p -> p m"))

        denom2 = small.tile([P, M_CH], FP32, tag="denom2", name="denom2")
        nc.vector.scalar_tensor_tensor(
            out=denom2, in0=rs_bb, scalar=-EXPD, in1=colsum_l,
            op0=ALU.add, op1=ALU.add,
        )

        nc.scalar.activation(out=denom1, in_=denom1, func=ACTF.Ln)
        nc.scalar.activation(out=denom2, in_=denom2, func=ACTF.Ln)
        nc.vector.tensor_add(denom1, denom1, denom2)  # ld1 + ld2

        combo = scratch.tile([P, M_CH], FP32, tag="combo", name="combo")
        ppart = small.tile([P, 1], FP32, tag="ppart", name="ppart")
        nc.vector.scalar_tensor_tensor(
            out=combo, in0=dab, scalar=-2.0 / TAU, in1=denom1,
            op0=ALU.mult, op1=ALU.add, accum_out=ppart,
        )
        lps = pcol.tile([1, 1], FP32, tag="col", name="lps")
        nc.tensor.matmul(lps, ones_f32, ppart, start=True, stop=True)
        lsb = small.tile([1, 1], FP32, tag="lsb", name="lsb")
        nc.scalar.mul(lsb, lps, 0.5 / N)

        nc.scalar.dma_start(out=cc2_in, in_=lsb)
        nc.gpsimd.collective_compute(
            "AllReduce",
            ALU.add,
            replica_groups=[list(range(NCORES))],
            ins=[cc2_in.opt()],
            outs=[cc2_out.opt()],
        )
        nc.scalar.dma_start(out=loss, in_=cc2_out)

    nc.compile()
    return nc


_NC_CACHE = None


def _get_nc():
    global _NC_CACHE
    if _NC_CACHE is None:
        _NC_CACHE = _build()
    return _NC_CACHE


def _in_maps(z1, z2):
    z1 = np.ascontiguousarray(np.asarray(z1), dtype=np.float32)
    z2 = np.ascontiguousarray(np.asarray(z2), dtype=np.float32)
    z1T = np.ascontiguousarray(z1.T)
    z2T = np.ascontiguousarray(z2.T)
    maps = []
    for c in range(NCORES):
        sl = slice(LOCAL * c, LOCAL * (c + 1))
        maps.append(
            {
                "z1": z1,
                "z2": z2,
                "z1T": z1T,
                "z2T": z2T,
                "z1l": np.ascontiguousarray(z1[sl]),
                "z2l": np.ascontiguousarray(z2[sl]),
                "z1lT": np.ascontiguousarray(z1T[:, sl]),
                "z2lT": np.ascontiguousarray(z2T[:, sl]),
            }
        )
    return maps


def kernel(z1, z2):
    nc = _get_nc()
    res = run_bass_kernel_spmd(nc, _in_maps(z1, z2), list(range(NCORES)))
    return np.asarray(res.results[0]["loss"], dtype=np.float32).reshape(())


def _install_ntff_hook_shim():
    """The agent image's antenv lacks axon_hooks; recreate the documented
    ctypes hook (same as trn_agent_boot.trn_boot._ntff_profile_via_ctypes)
    so run_bass_kernel_spmd(trace=True) can capture NTFF profiles."""
    import sys, types, ctypes, contextlib

    if "antenv.axon_hooks" in sys.modules:
        return
    so_path = "/opt/axon/libaxon_pjrt.so"
    lib = ctypes.CDLL(so_path)
    if not hasattr(lib, "axon_start_nrt_profile"):
        return
    lib.axon_start_nrt_profile.argtypes = [
        ctypes.POINTER(ctypes.c_int64),
        ctypes.c_size_t,
    ]
    lib.axon_start_nrt_profile.restype = ctypes.c_int64
    lib.axon_stop_nrt_profile.argtypes = [ctypes.c_char_p]
    lib.axon_stop_nrt_profile.restype = ctypes.c_int64

    @contextlib.contextmanager
    def _hook(output_dir, device_ids):
        import jax

        jax.devices()
        if device_ids:
            ids = (ctypes.c_int64 * len(device_ids))(*device_ids)
            rc = lib.axon_start_nrt_profile(ids, len(device_ids))
        else:
            rc = lib.axon_start_nrt_profile(None, 0)
        if rc != 0:
            raise RuntimeError(f"axon_start_nrt_profile rc={rc}")
        try:
            yield
        finally:
            n = lib.axon_stop_nrt_profile(str(output_dir).encode())
            if n < 0:
                raise RuntimeError(f"axon_stop_nrt_profile rc={n}")
            print(f"profile: {n} file(s) written to {output_dir}", file=sys.stderr)

    mod = types.ModuleType("antenv.axon_hooks")
    mod.get_axon_ntff_profile_hook = lambda: _hook
    mod.set_axon_ntff_profile_hook = lambda h: None
    sys.modules["antenv.axon_hooks"] = mod


def kernel_traced(z1, z2):
    """Same as kernel() but with NTFF profiling; returns (loss, exec_time_ns,
    trace_path)."""
    import concourse.bass_utils as bu

    _install_ntff_hook_shim()
    bu.upload_artifacts = lambda tmpdir: "local://" + tmpdir  # no egress
    nc = _get_nc()
    res = run_bass_kernel_spmd(
        nc, _in_maps(z1, z2), list(range(NCORES)), trace=True
    )
    out = np.asarray(res.results[0]["loss"], dtype=np.float32).reshape(())
    trace_path = (
        res.instructions_and_trace[1] if res.instructions_and_trace else None
    )
    return out, res.exec_time_ns, trace_path



# revision 6
# speedup vs baseline: 2.0502x; 2.0502x over previous
"""Contrastive loss (GRACE-style semi_loss pair) on 8 trn2 NeuronCores.

Math (reference):
    a = z1 / ||z1||_row ; b = z2 / ||z2||_row         (N=8192, D=512)
    refl    = exp(a @ a.T / tau) ; between = exp(a @ b.T / tau)
    l1_i = -log(between_ii / (refl.sum(1) + between.sum(1) - refl_ii))
    l2   = same with (z2, z1) swapped
    loss = mean(0.5 * (l1 + l2))

Identities:
  - between2 rowsums = COLUMN sums of exp(a@b.T/tau): one cross-core
    reduction of [8192] floats, no 4th matmul.
  - refl_ii = exp(1/tau) exactly; between_ii needs only dab_i = a_i . b_i.
  - l1_i + l2_i = beta_i + ln(denom2_i) with
    beta_i = ln(denom1_i) - 2 dab_i / tau.

Design (v2):
  - Per core inputs: z1T/z2T [512,8192] fp32 (shared, the only big reads),
    z1l/z2l row-major local slices (norms + dab), z1lT/z2lT (stationary),
    selp (per-core 8x64 selector for SPMD-positional alpha writes).
  - Norms: local sumsq on DVE + Newton rsqrt; 1/norm bf16 AllGathered
    (32KB) while zT streams; no full row-major z reads at all.
  - Matmuls in fp8e4 (x16-scaled operands) with DoubleRow perf mode:
    K=256 per instruction, 2 instrs per [128,512] product.
  - PSUM per m: one [128,1536] tile = aa|ab|bb. ACT does ONE fused
    exp+rowsum over aa|ab (denom1 needs only the sum) and exp over bb;
    bb rowsum on DVE. Column sums of exp(ab) accumulate on the PE via
    ones-matmuls, deferred one m-step so the PE never waits on ACT.
  - Tail: ONE AllReduce over [colsums+alpha(8192) | alpha-block(8192) |
    sum-beta(1)]: the AR itself sums partial colsums AND adds alpha_j
    (positioned at its global slot by a selector matmul) so AR output IS
    denom2; every core then computes the final scalar locally.
"""

import numpy as np
from contextlib import ExitStack

import concourse.bass as bass
import concourse.tile as tile
from concourse import bacc, mybir
from concourse.bass_utils import run_bass_kernel_spmd

N = 8192
D = 512
P = 128
NCORES = 8
LOCAL = N // NCORES            # 1024 rows per core
M_CH = LOCAL // P              # 8 local row blocks of 128
N_CH = N // 512                # 16 column chunks of 512
KC = D // P                    # 4 contraction chunks of 128
SUPW = 1024                    # DMA super-chunk width (2 chunks)
N_SUP = N // SUPW              # 8 supers
TAU = 0.4
EXPD = float(np.exp(1.0 / TAU))
Y0 = float(D) ** -0.5          # Newton rsqrt seed
FSC = 16.0                     # fp8 operand scale
ES = 1.0 / (FSC * FSC * TAU)   # exp scale on S' = 256*S

FP32 = mybir.dt.float32
BF16 = mybir.dt.bfloat16
FP8 = mybir.dt.float8e4
ALU = mybir.AluOpType
ACTF = mybir.ActivationFunctionType
DR = mybir.MatmulPerfMode.DoubleRow


def _build():
    nc = bacc.Bacc("TRN2", debug=False, num_devices=NCORES)
    z1T = nc.dram_tensor("z1T", [D, N], FP32, kind="ExternalInput").ap()
    z2T = nc.dram_tensor("z2T", [D, N], FP32, kind="ExternalInput").ap()
    z1l = nc.dram_tensor("z1l", [LOCAL, D], FP32, kind="ExternalInput").ap()
    z2l = nc.dram_tensor("z2l", [LOCAL, D], FP32, kind="ExternalInput").ap()
    z1lT = nc.dram_tensor("z1lT", [D, LOCAL], FP32, kind="ExternalInput").ap()
    z2lT = nc.dram_tensor("z2lT", [D, LOCAL], FP32, kind="ExternalInput").ap()
    selp = nc.dram_tensor("selp", [M_CH, 64], FP32, kind="ExternalInput").ap()
    loss = nc.dram_tensor("loss", [1, 1], FP32, kind="ExternalOutput").ap()

    with tile.TileContext(nc) as tc, ExitStack() as ctx:
        big = ctx.enter_context(tc.tile_pool(name="big", bufs=1))
        stg = ctx.enter_context(tc.tile_pool(name="stg", bufs=2))
        rowz = ctx.enter_context(tc.tile_pool(name="rowz", bufs=4))
        scr = ctx.enter_context(tc.tile_pool(name="scr", bufs=2))
        atp = ctx.enter_context(tc.tile_pool(name="atp", bufs=4))
        eabp = ctx.enter_context(tc.tile_pool(name="eabp", bufs=4))
        ebbp = ctx.enter_context(tc.tile_pool(name="ebbp", bufs=3))
        pmm = ctx.enter_context(tc.tile_pool(name="pmm", bufs=2, space="PSUM"))
        pbc = ctx.enter_context(tc.tile_pool(name="pbc", bufs=1, space="PSUM"))
        pcol = ctx.enter_context(tc.tile_pool(name="pcol", bufs=1, space="PSUM"))
        dram = ctx.enter_context(tc.tile_pool(name="dram", bufs=1, space="DRAM"))

        # ---- constants --------------------------------------------------
        ones_col = big.tile([P, 1], BF16, tag="ones_col", name="ones_col")
        nc.vector.memset(ones_col, 1.0)
        ones_f32 = big.tile([P, 1], FP32, tag="ones_f32", name="ones_f32")
        nc.vector.memset(ones_f32, 1.0)
        ones_row = big.tile([1, P], BF16, tag="ones_row", name="ones_row")
        nc.vector.memset(ones_row, 1.0)

        # ---- persistent tiles -------------------------------------------
        ATL1 = big.tile([P, KC, LOCAL], FP8, tag="ATL1", name="ATL1")
        ATL2 = big.tile([P, KC, LOCAL], FP8, tag="ATL2", name="ATL2")
        invnb1 = big.tile([P, LOCAL], BF16, tag="invnb1", name="invnb1")
        invnb2 = big.tile([P, LOCAL], BF16, tag="invnb2", name="invnb2")
        ivall = big.tile([1, 2 * N], BF16, tag="ivall", name="ivall")
        sel_sb = big.tile([M_CH, 64], FP32, tag="sel_sb", name="sel_sb")

        rsp1 = [
            big.tile([P, N_CH], FP32, tag=f"rsp1_{m}", name=f"rsp1_{m}")
            for m in range(M_CH)
        ]
        rsp2 = [
            big.tile([P, N_CH], FP32, tag=f"rsp2_{m}", name=f"rsp2_{m}")
            for m in range(M_CH)
        ]

        ss1 = big.tile([P, M_CH], FP32, tag="ss1", name="ss1")
        ss2 = big.tile([P, M_CH], FP32, tag="ss2", name="ss2")
        u_ab = big.tile([P, M_CH], FP32, tag="u_ab", name="u_ab")

        # collective buffers
        ag_in = dram.tile([1, 2 * LOCAL], BF16, tag="ag_in", name="ag_in")
        ag_out = dram.tile([1, 2 * N], BF16, tag="ag_out", name="ag_out")
        rs_in = dram.tile([1, 2 * N + 1], FP32, tag="rs_in", name="rs_in")
        rs_out = dram.tile(
            [1, 2 * N + 1], FP32, tag="rs_out", name="rs_out", addr_space="Shared"
        )

        GROUPS = [list(range(NCORES))]

        def sumsq(zt, acc_slice, nm, other=None):
            sq = scr.tile([P, D], BF16, tag="sq", name=f"sq_{nm}", bufs=2)
            nc.vector.scalar_tensor_tensor(
                out=sq, in0=zt, scalar=1.0,
                in1=other if other is not None else zt,
                op0=ALU.mult, op1=ALU.mult, accum_out=acc_slice,
            )

        def rsqrt_newton(ss, w, nm, iters=3):
            ssh = scr.tile([P, w], FP32, tag="rq_ssh", name=f"ssh_{nm}")
            nc.vector.tensor_scalar_mul(ssh, ss, 0.5)
            y = scr.tile([P, w], FP32, tag="rq_y", name=f"y_{nm}")
            nc.vector.tensor_scalar(
                out=y, in0=ssh, scalar1=-(Y0**3), scalar2=1.5 * Y0,
                op0=ALU.mult, op1=ALU.add,
            )
            t = scr.tile([P, w], FP32, tag="rq_t", name=f"t_{nm}")
            u = scr.tile([P, w], FP32, tag="rq_u", name=f"u_{nm}")
            for _ in range(iters - 1):
                nc.vector.tensor_mul(t, y, y)
                nc.vector.tensor_mul(t, t, ssh)
                nc.vector.tensor_mul(u, y, t)
                nc.vector.scalar_tensor_tensor(
                    out=y, in0=y, scalar=1.5, in1=u,
                    op0=ALU.mult, op1=ALU.subtract,
                )
            return y

        # ---- head DMAs (sync queue: local rows, stationary, supers) -----
        r1 = []
        r2 = []
        for t in range(M_CH):
            zt1 = rowz.tile([P, D], FP32, tag="r1", name=f"zl1_{t}")
            nc.sync.dma_start(out=zt1, in_=z1l[P * t : P * (t + 1), :])
            r1.append(zt1)
            zt2 = rowz.tile([P, D], FP32, tag="r2", name=f"zl2_{t}")
            nc.sync.dma_start(out=zt2, in_=z2l[P * t : P * (t + 1), :])
            r2.append(zt2)
        sl1 = big.tile([P, KC, LOCAL], FP32, tag="sl1", name="sl1")
        nc.sync.dma_start(
            out=sl1, in_=z1lT.rearrange("(k p) n -> p k n", p=P)
        )
        sl2 = big.tile([P, KC, LOCAL], FP32, tag="sl2", name="sl2")
        nc.sync.dma_start(
            out=sl2, in_=z2lT.rearrange("(k p) n -> p k n", p=P)
        )
        nc.scalar.dma_start(out=sel_sb, in_=selp)

        st1 = {}
        st2 = {}

        def prefetch(s):
            st1[s] = stg.tile([P, KC, SUPW], FP32, tag="st1", name=f"st1_{s}")
            nc.sync.dma_start(
                out=st1[s],
                in_=z1T.rearrange("(k p) n -> p k n", p=P)[
                    :, :, SUPW * s : SUPW * (s + 1)
                ],
            )
            st2[s] = stg.tile([P, KC, SUPW], FP32, tag="st2", name=f"st2_{s}")
            nc.sync.dma_start(
                out=st2[s],
                in_=z2T.rearrange("(k p) n -> p k n", p=P)[
                    :, :, SUPW * s : SUPW * (s + 1)
                ],
            )

        prefetch(0)
        prefetch(1)

        # ---- local norms -> AllGather (critical chain first) ------------
        # all three consumers of a row tile issue together so the rowz pool
        # (bufs=4) releases slots before later row DMAs need them
        for t in range(M_CH):
            sumsq(r1[t], ss1[:, t : t + 1], f"l1_{t}")
            sumsq(r2[t], ss2[:, t : t + 1], f"l2_{t}")
            sumsq(r1[t], u_ab[:, t : t + 1], f"u_{t}", other=r2[t])
        inv1 = rsqrt_newton(ss1, M_CH, "l1")
        inv2 = rsqrt_newton(ss2, M_CH, "l2")

        ivcl = scr.tile([P, 2 * M_CH], BF16, tag="ivcl", name="ivcl")
        nc.vector.tensor_copy(ivcl[:, 0:M_CH], inv1)
        nc.vector.tensor_copy(ivcl[:, M_CH : 2 * M_CH], inv2)
        nc.scalar.dma_start(
            out=ag_in[:, 0:LOCAL].rearrange("o (t p) -> p (o t)", p=P),
            in_=ivcl[:, 0:M_CH],
        )
        nc.scalar.dma_start(
            out=ag_in[:, LOCAL : 2 * LOCAL].rearrange("o (t p) -> p (o t)", p=P),
            in_=ivcl[:, M_CH : 2 * M_CH],
        )
        # local 1/norm broadcasts (issued BEFORE the AG-gated ivall load so
        # the scalar DMA queue never head-of-line blocks on the collective)
        nc.scalar.dma_start(
            out=invnb1, in_=ag_in[:, 0:LOCAL].to_broadcast([P, LOCAL])
        )
        nc.scalar.dma_start(
            out=invnb2, in_=ag_in[:, LOCAL : 2 * LOCAL].to_broadcast([P, LOCAL])
        )
        nc.gpsimd.collective_compute(
            "AllGather",
            ALU.bypass,
            replica_groups=GROUPS,
            ins=[ag_in.opt()],
            outs=[ag_out.opt()],
        )
        nc.scalar.dma_start(out=ivall, in_=ag_out)

        # dab + stationary fp8 operands (off the AG critical path)
        dab = big.tile([P, M_CH], FP32, tag="dab", name="dab")
        nc.vector.tensor_mul(dab, u_ab, inv1)
        nc.vector.tensor_mul(dab, dab, inv2)

        for k in range(KC):
            nc.vector.scalar_tensor_tensor(
                out=ATL1[:, k, :], in0=sl1[:, k, :], scalar=FSC, in1=invnb1,
                op0=ALU.mult, op1=ALU.mult,
            )
            nc.vector.scalar_tensor_tensor(
                out=ATL2[:, k, :], in0=sl2[:, k, :], scalar=FSC, in1=invnb2,
                op0=ALU.mult, op1=ALU.mult,
            )

        # ---- main loop --------------------------------------------------
        AT1 = {}
        AT2 = {}

        def prep(n):
            """Broadcast 1/norms for chunk n and scale zT slices to fp8."""
            s, h = n // 2, n % 2
            off = 512 * h
            AT1[n] = atp.tile([P, KC, 512], FP8, tag="AT1", name=f"AT1_{n}")
            AT2[n] = atp.tile([P, KC, 512], FP8, tag="AT2", name=f"AT2_{n}")
            # rows 512n..512(n+1) belong to core cblk = n//2, half h; the
            # AllGathered layout per core block is [inv1(1024) | inv2(1024)]
            cblk = n // 2
            base = 2 * LOCAL * cblk
            iv1 = ivall[0:1, base + 512 * h : base + 512 * h + 512]
            iv2 = ivall[0:1, base + LOCAL + 512 * h : base + LOCAL + 512 * h + 512]
            pb1 = pbc.tile([P, 512], FP32, tag="pb", name=f"pb1_{n}")
            nc.tensor.matmul(pb1, ones_row, iv1, start=True, stop=True)
            pbb1 = scr.tile([P, 512], BF16, tag="pbb1", name=f"pbb1_{n}")
            nc.vector.tensor_copy(pbb1, pb1)
            pb2 = pbc.tile([P, 512], FP32, tag="pb", name=f"pb2_{n}")
            nc.tensor.matmul(pb2, ones_row, iv2, start=True, stop=True)
            pbb2 = scr.tile([P, 512], BF16, tag="pbb2", name=f"pbb2_{n}")
            nc.vector.tensor_copy(pbb2, pb2)
            for k in range(KC):
                nc.vector.scalar_tensor_tensor(
                    out=AT1[n][:, k, :], in0=st1[s][:, k, off : off + 512],
                    scalar=FSC, in1=pbb1, op0=ALU.mult, op1=ALU.mult,
                )
            for k in range(KC):
                nc.vector.scalar_tensor_tensor(
                    out=AT2[n][:, k, :], in0=st2[s][:, k, off : off + 512],
                    scalar=FSC, in1=pbb2, op0=ALU.mult, op1=ALU.mult,
                )

        # deferred column-sum state: (colp_tile, eab_tile, m_index, n)
        pend = []
        colp = {}
        csb_t = {}

        def flush_colsum():
            """Issue the ones-matmul for the oldest pending exp(ab) tile."""
            if not pend:
                return
            n, m, eab = pend.pop(0)
            if m == 0:
                colp[n] = pcol.tile([1, 512], FP32, tag="col", name=f"colp_{n}")
            nc.tensor.matmul(
                colp[n], ones_col, eab[:, 512:1024],
                start=(m == 0), stop=(m == M_CH - 1),
            )
            if m == M_CH - 1:
                csb = scr.tile([1, 512], FP32, tag="csb", name=f"csb_{n}")
                nc.vector.tensor_copy(csb, colp[n])
                nc.scalar.dma_start(
                    out=rs_in[:, 512 * n : 512 * (n + 1)], in_=csb
                )

        def main_chunk(n):
            for m in range(M_CH):
                mm = pmm.tile([P, 1536], FP32, tag="mm", name=f"mm_{n}_{m}")
                lhs1 = ATL1[:, :, P * m : P * (m + 1)]
                lhs2 = ATL2[:, :, P * m : P * (m + 1)]
                for half, (lo, hi) in enumerate(((0, 2), (2, 4))):
                    nc.tensor.matmul(
                        mm[:, 0:512], lhs1[:, lo:hi, :], AT1[n][:, lo:hi, :],
                        start=(half == 0), stop=(half == 1), perf_mode=DR,
                    )
                for half, (lo, hi) in enumerate(((0, 2), (2, 4))):
                    nc.tensor.matmul(
                        mm[:, 512:1024], lhs1[:, lo:hi, :], AT2[n][:, lo:hi, :],
                        start=(half == 0), stop=(half == 1), perf_mode=DR,
                    )
                for half, (lo, hi) in enumerate(((0, 2), (2, 4))):
                    nc.tensor.matmul(
                        mm[:, 1024:1536], lhs2[:, lo:hi, :], AT2[n][:, lo:hi, :],
                        start=(half == 0), stop=(half == 1), perf_mode=DR,
                    )
                # deferred colsum matmul: never blocks the PE on ACT
                flush_colsum()
                eab = eabp.tile([P, 1024], BF16, tag="eab", name=f"eab_{n}_{m}")
                nc.scalar.activation(
                    out=eab, in_=mm[:, 0:1024], func=ACTF.Exp, scale=ES,
                    accum_out=rsp1[m][:, n : n + 1],
                )
                ebb = ebbp.tile([P, 512], BF16, tag="ebb", name=f"ebb_{n}_{m}")
                nc.scalar.activation(
                    out=ebb, in_=mm[:, 1024:1536], func=ACTF.Exp, scale=ES,
                )
                nc.vector.reduce_sum(
                    out=rsp2[m][:, n : n + 1], in_=ebb, axis=mybir.AxisListType.X
                )
                pend.append((n, m, eab))

        # software pipeline: operand prep one chunk ahead, supers two ahead
        prep(0)
        prep(1)
        for n in range(N_CH):
            if n % 2 == 0 and n // 2 + 2 < N_SUP:
                prefetch(n // 2 + 2)
            if n + 2 < N_CH:
                prep(n + 2)
            main_chunk(n)
        while pend:
            flush_colsum()

        # ---- tail -------------------------------------------------------
        rs1 = big.tile([P, M_CH], FP32, tag="rs1", name="rs1")
        rs2 = big.tile([P, M_CH], FP32, tag="rs2", name="rs2")
        for m in range(M_CH):
            nc.vector.reduce_sum(
                out=rs1[:, m : m + 1], in_=rsp1[m], axis=mybir.AxisListType.X
            )
            nc.vector.reduce_sum(
                out=rs2[:, m : m + 1], in_=rsp2[m], axis=mybir.AxisListType.X
            )

        # beta = ln(denom1) - 2 dab / tau ; sum over local rows
        denom1 = scr.tile([P, M_CH], FP32, tag="denom1", name="denom1")
        nc.vector.tensor_scalar_add(denom1, rs1, -EXPD)
        nc.scalar.activation(out=denom1, in_=denom1, func=ACTF.Ln)
        combo = scr.tile([P, M_CH], FP32, tag="combo", name="combo")
        ppart = big.tile([P, 1], FP32, tag="ppart", name="ppart")
        nc.vector.scalar_tensor_tensor(
            out=combo, in0=dab, scalar=-2.0 / TAU, in1=denom1,
            op0=ALU.mult, op1=ALU.add, accum_out=ppart,
        )
        lps = pcol.tile([1, 512], FP32, tag="col", name="lps")
        nc.tensor.matmul(lps[0:1, 0:1], ones_f32, ppart, start=True, stop=True)
        lsb = big.tile([1, 1], FP32, tag="lsb", name="lsb")
        nc.vector.tensor_copy(lsb, lps[0:1, 0:1])
        nc.scalar.dma_start(out=rs_in[:, 2 * N : 2 * N + 1], in_=lsb)

        # alpha = rs2 - EXPD, positioned at global row slot via selector
        alpha = scr.tile([P, M_CH], FP32, tag="alpha", name="alpha")
        nc.vector.tensor_scalar_add(alpha, rs2, -EXPD)
        alr = dram.tile([1, LOCAL], FP32, tag="alr", name="alr")
        nc.scalar.dma_start(
            out=alr.rearrange("o (t p) -> p (o t)", p=P), in_=alpha
        )
        alT = big.tile([M_CH, P], FP32, tag="alT", name="alT")
        nc.scalar.dma_start(
            out=alT, in_=alr.rearrange("o (t p) -> t (o p)", p=P)
        )
        alf = pmm.tile([P, 1536], FP32, tag="mm", name="alf")
        nc.tensor.matmul(alf[0:64, 0:P], sel_sb, alT, start=True, stop=True)
        af_sb = big.tile([64, P], FP32, tag="af_sb", name="af_sb")
        nc.vector.tensor_copy(af_sb, alf[0:64, 0:P])
        nc.scalar.dma_start(
            out=rs_in[:, N : 2 * N].rearrange("o (t p) -> t (o p)", p=P),
            in_=af_sb,
        )

        nc.gpsimd.collective_compute(
            "AllReduce",
            ALU.add,
            replica_groups=GROUPS,
            ins=[rs_in.opt()],
            outs=[rs_out.opt()],
        )

        # final scalar: every core computes it (SPMD); core 0's is read
        cs_t = big.tile([P, 64], FP32, tag="cs_t", name="cs_t")
        nc.scalar.dma_start(
            out=cs_t, in_=rs_out[:, 0:N].rearrange("o (t p) -> p (o t)", p=P)
        )
        al_t = big.tile([P, 64], FP32, tag="al_t", name="al_t")
        nc.scalar.dma_start(
            out=al_t, in_=rs_out[:, N : 2 * N].rearrange("o (t p) -> p (o t)", p=P)
        )
        sb_t = big.tile([1, 1], FP32, tag="sb_t", name="sb_t")
        nc.scalar.dma_start(out=sb_t, in_=rs_out[:, 2 * N : 2 * N + 1])

        dn2 = big.tile([P, 64], FP32, tag="dn2", name="dn2")
        nc.vector.tensor_add(dn2, cs_t, al_t)
        nc.scalar.activation(out=dn2, in_=dn2, func=ACTF.Ln)
        lnp = big.tile([P, 1], FP32, tag="lnp", name="lnp")
        nc.vector.reduce_sum(out=lnp, in_=dn2, axis=mybir.AxisListType.X)
        tl2 = pcol.tile([1, 512], FP32, tag="col", name="tl2")
        nc.tensor.matmul(tl2[0:1, 0:1], ones_f32, lnp, start=True, stop=True)
        tot = big.tile([1, 1], FP32, tag="tot", name="tot")
        nc.vector.tensor_add(tot, tl2[0:1, 0:1], sb_t)
        nc.scalar.mul(tot, tot, 0.5 / N)
        nc.scalar.dma_start(out=loss, in_=tot)

    nc.compile()
    return nc


_NC_CACHE = None


def _get_nc():
    global _NC_CACHE
    if _NC_CACHE is None:
        _NC_CACHE = _build()
    return _NC_CACHE


def _in_maps(z1, z2):
    z1 = np.ascontiguousarray(np.asarray(z1), dtype=np.float32)
    z2 = np.ascontiguousarray(np.asarray(z2), dtype=np.float32)
    z1T = np.ascontiguousarray(z1.T)
    z2T = np.ascontiguousarray(z2.T)
    maps = []
    for c in range(NCORES):
        sl = slice(LOCAL * c, LOCAL * (c + 1))
        sel = np.zeros((M_CH, 64), dtype=np.float32)
        for i in range(M_CH):
            sel[i, M_CH * c + i] = 1.0
        maps.append(
            {
                "z1T": z1T,
                "z2T": z2T,
                "z1l": np.ascontiguousarray(z1[sl]),
                "z2l": np.ascontiguousarray(z2[sl]),
                "z1lT": np.ascontiguousarray(z1T[:, sl]),
                "z2lT": np.ascontiguousarray(z2T[:, sl]),
                "selp": sel,
            }
        )
    return maps


def kernel(z1, z2):
    nc = _get_nc()
    res = run_bass_kernel_spmd(nc, _in_maps(z1, z2), list(range(NCORES)))
    return np.asarray(res.results[0]["loss"], dtype=np.float32).reshape(())


def _install_ntff_hook_shim():
    """The agent image's antenv lacks axon_hooks; recreate the documented
    ctypes hook (same as trn_agent_boot.trn_boot._ntff_profile_via_ctypes)
    so run_bass_kernel_spmd(trace=True) can capture NTFF profiles."""
    import sys, types, ctypes, contextlib

    if "antenv.axon_hooks" in sys.modules:
        return
    so_path = "/opt/axon/libaxon_pjrt.so"
    lib = ctypes.CDLL(so_path)
    if not hasattr(lib, "axon_start_nrt_profile"):
        return
    lib.axon_start_nrt_profile.argtypes = [
        ctypes.POINTER(ctypes.c_int64),
        ctypes.c_size_t,
    ]
    lib.axon_start_nrt_profile.restype = ctypes.c_int64
    lib.axon_stop_nrt_profile.argtypes = [ctypes.c_char_p]
    lib.axon_stop_nrt_profile.restype = ctypes.c_int64

    @contextlib.contextmanager
    def _hook(output_dir, device_ids):
        import jax

        jax.devices()
        if device_ids:
            ids = (ctypes.c_int64 * len(device_ids))(*device_ids)
            rc = lib.axon_start_nrt_profile(ids, len(device_ids))
        else:
            rc = lib.axon_start_nrt_profile(None, 0)
        if rc != 0:
            raise RuntimeError(f"axon_start_nrt_profile rc={rc}")
        try:
            yield
        finally:
            n = lib.axon_stop_nrt_profile(str(output_dir).encode())
            if n < 0:
                raise RuntimeError(f"axon_stop_nrt_profile rc={n}")
            print(f"profile: {n} file(s) written to {output_dir}", file=sys.stderr)

    mod = types.ModuleType("antenv.axon_hooks")
    mod.get_axon_ntff_profile_hook = lambda: _hook
    mod.set_axon_ntff_profile_hook = lambda h: None
    sys.modules["antenv.axon_hooks"] = mod


def kernel_traced(z1, z2):
    """Same as kernel() but with NTFF profiling; returns (loss, exec_time_ns,
    trace_path)."""
    import concourse.bass_utils as bu

    _install_ntff_hook_shim()
    bu.upload_artifacts = lambda tmpdir: "local://" + tmpdir  # no egress
    nc = _get_nc()
    res = run_bass_kernel_spmd(
        nc, _in_maps(z1, z2), list(range(NCORES)), trace=True
    )
    out = np.asarray(res.results[0]["loss"], dtype=np.float32).reshape(())
    trace_path = (
        res.instructions_and_trace[1] if res.instructions_and_trace else None
    )
    return out, res.exec_time_ns, trace_path
